# revision 2
# baseline (speedup 1.0000x reference)
"""Trainium2 Bass kernel for the LGP-instruction module (read -> op bank -> write).

Data-parallel over batch: core b computes x[b] (2048, 4096).
All HBM traffic is bf16 (x, weights, output) with fp32 PSUM accumulation:
~36 MB/core instead of ~72 MB fp32, so the DMA roofline halves to ~100 us.
The two linear ops of the bank (identity, negation) are folded into one
effective weight matrix on the host: 7 op matmuls instead of 8.

Device pipeline per core and T-chunk of 512:
  phase 1: valuesT[C, T] = sum_vt rw_tile[vt].T @ xT_tile[vt]   (PSUM, fp32)
  phase 2: h_i = W_i.T @ valuesT (PSUM) -> ACT f_i(h + b_i) -> DVE
           weighted-accumulate (fp32, final op writes bf16)
  phase 3: out[Tsub, V] = accT.T @ wwT -> drain -> DMA store (bf16)
Host prep: read_w softmax, write_w*out_scale transpose, x[b].T layout,
all cast to bf16; output cast back to fp32 on host.
"""
import sys
import numpy as np

if '/opt/trn_rl_repo' not in sys.path:
    sys.path.insert(0, '/opt/trn_rl_repo')

B, T, V, C = 8, 2048, 4096, 128
NCORES = 8
NV = V // 128     # 32 v-tiles
NTC = T // 512    # 4 T-chunks
NOP = 7           # op bank after folding the two linear ops

_CACHE = {}
LAST_RESULT = None


def _build(pre, post):
    from concourse import bass, bacc, tile, mybir
    f32, bf16 = mybir.dt.float32, mybir.dt.bfloat16
    AF = mybir.ActivationFunctionType
    ts = bass.ts
    FUNCS = [AF.Identity, AF.Relu, AF.Gelu, AF.Square,
             AF.Abs, AF.Tanh, AF.Sigmoid]

    nc = bacc.Bacc("TRN2", target_bir_lowering=False, debug=False,
                   num_devices=NCORES)
    xT = nc.dram_tensor("xT", [V, T], bf16, kind="ExternalInput")
    rw = nc.dram_tensor("rw", [V, C], bf16, kind="ExternalInput")
    wwT = nc.dram_tensor("wwT", [C, V], bf16, kind="ExternalInput")
    opw = nc.dram_tensor("opw", [NOP, C, C], bf16, kind="ExternalInput")
    opb = nc.dram_tensor("opb", [C, NOP], f32, kind="ExternalInput")
    out = nc.dram_tensor("out", [T, V], bf16, kind="ExternalOutput")

    NBLK = 4          # xT load blocks per T-chunk
    VB = NV // NBLK   # 8 v-tiles per block

    # xT viewed as [p, vtile, t]
    xTr = xT.ap().rearrange("(vb p) t -> p vb t", p=128)

    with tile.TileContext(nc) as tc:
        with tc.tile_pool(name="const", bufs=1) as constp, \
             tc.tile_pool(name="xt", bufs=6) as xtp, \
             tc.tile_pool(name="vals_ps", bufs=2, space="PSUM") as vpsp, \
             tc.tile_pool(name="vals_sb", bufs=2) as vsbp, \
             tc.tile_pool(name="h_ps", bufs=3, space="PSUM") as hpsp, \
             tc.tile_pool(name="t_sb", bufs=3) as tp, \
             tc.tile_pool(name="acc", bufs=2) as accp, \
             tc.tile_pool(name="out_ps", bufs=3, space="PSUM") as opsp, \
             tc.tile_pool(name="out_sb", bufs=2) as osbp:

            rw_t = constp.tile([128, NV, C], bf16)
            nc.sync.dma_start(rw_t[:], rw.ap().rearrange("(vt p) c -> p vt c", p=128))
            wwT_t = constp.tile([C, V], bf16)
            nc.sync.dma_start(wwT_t[:], wwT.ap())
            opw_t = constp.tile([C, NOP, C], bf16)
            nc.sync.dma_start(opw_t[:], opw.ap().rearrange("i p c -> p i c"))
            opb_t = constp.tile([C, NOP], f32)
            nc.sync.dma_start(opb_t[:], opb.ap())

            for tcn in range(NTC):
                # read: accumulate over all V into one psum bank
                values = vpsp.tile([128, 512], f32)
                for blk in range(NBLK):
                    xt = xtp.tile([128, VB, 512], bf16)
                    nc.sync.dma_start(
                        xt[:], xTr[:, ts(blk, VB), ts(tcn, 512)])
                    for j in range(VB):
                        vt = blk * VB + j
                        nc.tensor.matmul(values[:], rw_t[:, vt, :], xt[:, j, :],
                                         start=(vt == 0), stop=(vt == NV - 1))
                vals = vsbp.tile([128, 512], bf16)
                nc.vector.tensor_copy(vals[:], values[:])

                # op bank: acc accumulates fp32; last op writes bf16 for matmul
                acc = accp.tile([128, 512], f32)
                acc_bf = accp.tile([128, 512], bf16)
                for i in range(NOP):
                    h = hpsp.tile([128, 512], f32)
                    nc.tensor.matmul(h[:], opw_t[:, i, :], vals[:],
                                     start=True, stop=True)
                    if i == 0:
                        nc.scalar.activation(acc[:], h[:], FUNCS[0],
                                             bias=opb_t[:, 0:1], scale=pre[0])
                    else:
                        t = tp.tile([128, 512], bf16)
                        nc.scalar.activation(t[:], h[:], FUNCS[i],
                                             bias=opb_t[:, i:i + 1], scale=pre[i])
                        dst = acc_bf if i == NOP - 1 else acc
                        nc.vector.scalar_tensor_tensor(
                            dst[:], t[:], post[i], acc[:],
                            op0=mybir.AluOpType.mult, op1=mybir.AluOpType.add)

                # write: out rows, stores on SWDGE so loads never queue behind them
                for sub in range(4):
                    osb = osbp.tile([128, V], bf16)
                    for nn in range(8):
                        ops_ = opsp.tile([128, 512], f32)
                        nc.tensor.matmul(ops_[:], acc_bf[:, ts(sub, 128)],
                                         wwT_t[:, ts(nn, 512)],
                                         start=True, stop=True)
                        idx = (tcn * 4 + sub) * 8 + nn
                        if idx % 9 < 2:   # ~2/9 of psum-drain copies go to ACT
                            nc.scalar.copy(osb[:, ts(nn, 512)], ops_[:])
                        else:
                            nc.vector.tensor_copy(osb[:, ts(nn, 512)], ops_[:])
                    nc.gpsimd.dma_start(out.ap()[ts(tcn * 4 + sub, 128), :], osb[:])
    nc.compile()
    return nc


def _softmax(x, axis):
    x = np.asarray(x, np.float32)
    m = x.max(axis=axis, keepdims=True)
    e = np.exp(x - m)
    return e / e.sum(axis=axis, keepdims=True)


def _to_bf16(a):
    """Round-to-nearest-even fp32 -> bf16, fast numpy bit twiddle."""
    import ml_dtypes
    a = np.ascontiguousarray(a, np.float32)
    u = a.view(np.uint32)
    r = ((u >> 16) & 1) + np.uint32(0x7FFF)
    return ((u + r) >> 16).astype(np.uint16).view(ml_dtypes.bfloat16)


def kernel(x, basis, read_coeffs, write_coeffs, op_logits, op_weights,
           op_biases, out_scale):
    global LAST_RESULT
    from concourse.bass_utils import run_bass_kernel_spmd

    x = np.asarray(x, np.float32)
    basis = np.asarray(basis, np.float32)
    read_coeffs = np.asarray(read_coeffs, np.float32)
    write_coeffs = np.asarray(write_coeffs, np.float32)
    op_logits = np.asarray(op_logits, np.float32)
    op_weights = np.asarray(op_weights, np.float64)
    op_biases = np.asarray(op_biases, np.float64)
    out_scale = np.float32(out_scale)

    read_w = _softmax(basis @ read_coeffs.T, axis=0)               # (V, C)
    wwT = np.ascontiguousarray((basis @ write_coeffs.T).T) * out_scale  # (C, V)
    w = _softmax(op_logits, axis=0).astype(np.float64)

    # device op order: [linear(0&4 folded), relu, gelu, square, abs, tanh, sigmoid]
    # fold the mixture weight into ACT scale/bias where the nonlinearity allows
    W_lin = w[0] * op_weights[0] - w[4] * op_weights[4]
    b_lin = w[0] * op_biases[0] - w[4] * op_biases[4]
    opw = np.stack([W_lin, op_weights[1], op_weights[2], op_weights[3],
                    op_weights[5], op_weights[6], op_weights[7]])
    pre = [1.0, w[1], 1.0, np.sqrt(w[3]), w[5], 1.0, 1.0]
    post = [1.0, 1.0, w[2], 1.0, 1.0, w[6], w[7]]
    pre = [float(v) for v in pre]
    post = [float(v) for v in post]
    opb = np.stack([b_lin, w[1] * op_biases[1], op_biases[2],
                    np.sqrt(w[3]) * op_biases[3], w[5] * op_biases[5],
                    op_biases[6], op_biases[7]], axis=1)  # (C, NOP)

    key = tuple(pre) + tuple(post)
    if key not in _CACHE:
        _CACHE[key] = _build(pre, post)
    nc = _CACHE[key]

    shared = {
        "rw": _to_bf16(read_w),
        "wwT": _to_bf16(wwT),
        "opw": _to_bf16(opw.astype(np.float32)),
        "opb": np.ascontiguousarray(opb, np.float32),
    }
    in_maps = []
    for b in range(B):
        m = dict(shared)
        m["xT"] = _to_bf16(np.ascontiguousarray(x[b].T))
        in_maps.append(m)

    res = run_bass_kernel_spmd(nc, in_maps, core_ids=list(range(NCORES)))
    LAST_RESULT = res
    out = np.empty((B, T, V), np.float32)
    for b in range(B):
        out[b] = np.asarray(res.results[b]["out"], np.float32)
    return out
